# revision 14
# baseline (speedup 1.0000x reference)
"""MoE (8 experts, top-2, shared expert) Trainium2 kernel.

Expert-parallel over 8 NeuronCores. The host performs only the dispatch
decision (top-2 expert ids -> compact per-expert token lists) and data
layout (transposes/gathers); all floating-point model math — router
logits, gates, expert SwiGLU, shared expert, and the cross-core combine
(ReduceScatter) — runs on device in fp32r matmuls with fp32 accumulation.

Device program per core (SPMD, identical program, per-core data):
  D1: hts[176, T] = silu(sw1_slice @ x) * (sw3_slice @ x)  (all tokens)
  A:  router logits for compact tokens (matmul) * validity mask -> gates
  B:  ht[I, C] = silu(w1 @ xg) * (w3 @ xg)   (compact tokens)
  C:  y[ct] = gate * (ht.T @ w2t) -> eacc[C, H] (dense write, compact order)
  D2: acc[t] = hts.T @ sw2_slice + eacc[inv_idx[t]]   (indirect GATHER with
      zero-row sentinel for tokens not routed to this core)
  E:  ReduceScatter(add) over acc -> this core's 256-token output slice
"""

import numpy as np

H = 1024          # hidden
I = 1408          # moe intermediate
E = 8             # experts == cores
T = 2048          # tokens (2*1024)
TOPK = 2
C = 640           # compact per-expert token capacity (max observed 540)
CH = 320          # ht token chunk (2 chunks; >=256 keeps fp32r at full rate)
ILOC = I // E     # 176: shared-expert intermediate slice per core
TSL = T // E      # 256: output token slice per core
KT = H // 128     # 8 contraction tiles over H
IT = I // 128     # 11 tiles over I
CT = C // 128     # 5 compact token tiles
TT = T // 128     # 16 token tiles
SIP = (128, ILOC - 128)   # shared I-slice partition tiles: 128 + 48
NCORES = 8
DTYPE = "f32r"     # "f32r" (full precision-ish) or "bf16" (faster DMA)

_BUILD_CACHE = {}


def _build(reps=1, use_cc=True, dtype=None, cap=None):
    lean = cap is not None and cap > C
    import concourse.bacc as bacc
    import concourse.bass as bass
    import concourse.mybir as mybir
    from concourse import tile
    from contextlib import ExitStack

    f32 = mybir.dt.float32
    f32r = mybir.dt.float32r
    i32 = mybir.dt.int32
    dt_mm = mybir.dt.bfloat16 if (dtype or DTYPE) == "bf16" else f32r
    AF = mybir.ActivationFunctionType
    MUL = mybir.AluOpType.mult

    C_ = cap or C
    CT_ = C_ // 128
    n_ch = max(1, (C_ + 511) // 512)
    CH_ = C_ // n_ch
    assert CH_ * n_ch == C_ and CH_ % 64 == 0, (C_, CH_)

    nc = bacc.Bacc("TRN2", target_bir_lowering=False, debug=False,
                   num_devices=NCORES)

    xg = nc.declare_dram_parameter("xg", [H, C_], f32r, isOutput=False)
    xt = nc.declare_dram_parameter("xt", [H, T], dt_mm, isOutput=False)
    w1t = nc.declare_dram_parameter("w1t", [IT, H, 128], dt_mm, isOutput=False)
    w3t = nc.declare_dram_parameter("w3t", [IT, H, 128], dt_mm, isOutput=False)
    w2t = nc.declare_dram_parameter("w2t", [I, H], dt_mm, isOutput=False)
    s1t = nc.declare_dram_parameter("s1t", [H, ILOC], dt_mm, isOutput=False)
    s3t = nc.declare_dram_parameter("s3t", [H, ILOC], dt_mm, isOutput=False)
    s2t = nc.declare_dram_parameter("s2t", [ILOC, H], dt_mm, isOutput=False)
    rwe = nc.declare_dram_parameter("rwe", [H, 16], f32r, isOutput=False)
    invi = nc.declare_dram_parameter("invi", [T, 1], i32, isOutput=False)
    msk = nc.declare_dram_parameter("msk", [128, CT_], f32, isOutput=False)
    out = nc.declare_dram_parameter("out", [TSL, H], f32, isOutput=True)

    acc = nc.dram_tensor("acc", [T, H], f32)
    eacc = nc.dram_tensor("eacc", [C_ + 128, H], f32)
    rs_out = nc.dram_tensor("rs_out", [TSL, H], f32)

    with tile.TileContext(nc) as tc, ExitStack() as ctx:
        sres = ctx.enter_context(tc.tile_pool(name="sres", bufs=1))
        wstr = ctx.enter_context(tc.tile_pool(name="wstr",
                                              bufs=1 if lean else 2))
        xstr = ctx.enter_context(tc.tile_pool(name="xstr",
                                              bufs=1 if lean else 2))
        work = ctx.enter_context(tc.tile_pool(name="work", bufs=2))
        psA = ctx.enter_context(tc.tile_pool(name="psA", bufs=2, space="PSUM"))
        psB = ctx.enter_context(tc.tile_pool(name="psB", bufs=2, space="PSUM"))
        psY = ctx.enter_context(tc.tile_pool(name="psY", bufs=3, space="PSUM"))

        TCH = 256
        for _rep in range(reps):
            # ---- resident loads ----
            s13_sb = sres.tile([128, 2 * KT * ILOC], dt_mm, tag="s13_sb",
                               name="s13_sb")
            for k in range(KT):
                nc.sync.dma_start(s13_sb[:, k * ILOC:(k + 1) * ILOC],
                                  s1t[k * 128:(k + 1) * 128, :])
                nc.sync.dma_start(
                    s13_sb[:, (KT + k) * ILOC:(KT + k + 1) * ILOC],
                    s3t[k * 128:(k + 1) * 128, :])
            xg_sb = sres.tile([128, KT * C_], f32r, tag="xg_sb", name="xg_sb")
            for k in range(KT):
                nc.sync.dma_start(xg_sb[:, k * C_:(k + 1) * C_],
                                  xg[k * 128:(k + 1) * 128, :])
            rwe_sb = sres.tile([128, KT * 16], f32r, tag="rwe_sb",
                               name="rwe_sb")
            if dt_mm is f32r:
                xgb_sb = xg_sb
            else:
                xgb_sb = sres.tile([128, KT * C_], dt_mm, tag="xgb_sb",
                                   name="xgb_sb")
                for k in range(KT):
                    nc.vector.tensor_copy(xgb_sb[:, k * C_:(k + 1) * C_],
                                          xg_sb[:, k * C_:(k + 1) * C_])
            nc.sync.dma_start(rwe_sb[:],
                              rwe.rearrange("(k p) o -> p k o", p=128))
            invi_sb = sres.tile([128, TT], i32, tag="invi_sb", name="invi_sb")
            nc.sync.dma_start(invi_sb[:],
                              invi.rearrange("(c p) o -> p c o", p=128))
            msk_sb = sres.tile([128, CT_], f32, tag="msk_sb", name="msk_sb")
            nc.sync.dma_start(msk_sb[:], msk[:, :])
            s2_sb = sres.tile([128, 2 * H], dt_mm, tag="s2_sb", name="s2_sb")
            nc.sync.dma_start(s2_sb[:, 0:H], s2t[0:128, :])
            nc.sync.dma_start(s2_sb[:SIP[1], H:2 * H], s2t[128:ILOC, :])
            # zero sentinel row block for the combine gather
            ztile = work.tile([128, H], f32, tag="ztile", name="ztile",
                              bufs=1)
            nc.gpsimd.memset(ztile[:], 0.0)
            nc.sync.dma_start(eacc[C_:C_ + 128, :], ztile[:])

            # ---- D1: shared-expert hts[176, T] over all tokens ----
            hts = sres.tile([128, 2 * T], dt_mm, tag="hts", name="hts")
            for tt in range(T // TCH):
                xc = xstr.tile([128, KT * TCH], dt_mm, tag="xc", name="xc")
                for k in range(KT):
                    nc.sync.dma_start(
                        xc[:, k * TCH:(k + 1) * TCH],
                        xt[k * 128:(k + 1) * 128, tt * TCH:(tt + 1) * TCH])
                for si in range(2):
                    sip = SIP[si]
                    psa = psA.tile([128, TCH], f32, tag="a", name="psa_s",
                                   space="PSUM")
                    psb = psB.tile([128, TCH], f32, tag="b", name="psb_s",
                                   space="PSUM")
                    for k in range(KT):
                        nc.tensor.matmul(
                            psa[:sip, :],
                            lhsT=s13_sb[:, k * ILOC + si * 128:
                                        k * ILOC + si * 128 + sip],
                            rhs=xc[:, k * TCH:(k + 1) * TCH],
                            start=(k == 0), stop=(k == KT - 1))
                    for k in range(KT):
                        nc.tensor.matmul(
                            psb[:sip, :],
                            lhsT=s13_sb[:, (KT + k) * ILOC + si * 128:
                                        (KT + k) * ILOC + si * 128 + sip],
                            rhs=xc[:, k * TCH:(k + 1) * TCH],
                            start=(k == 0), stop=(k == KT - 1))
                    sact = work.tile([128, TCH], f32, tag="sact_s",
                                     name="sact_s",
                                     bufs=1 if lean else None)
                    nc.scalar.activation(sact[:sip, :], psa[:sip, :], AF.Silu)
                    nc.vector.tensor_tensor(
                        out=hts[:sip, si * T + tt * TCH:
                                si * T + (tt + 1) * TCH],
                        in0=sact[:sip, :], in1=psb[:sip, :], op=MUL)

            # ---- Part A: logits for compact tokens -> gates ----
            gates_sb = sres.tile([128, CT_], f32, tag="gates_sb",
                                 name="gates_sb")
            for ct in range(CT_):
                psl = psY.tile([128, 512], f32, tag="y", name="psl",
                               space="PSUM")
                for k in range(KT):
                    nc.tensor.matmul(
                        psl[:, 0:16],
                        lhsT=xg_sb[:, k * C_ + ct * 128: k * C_ + (ct + 1) * 128],
                        rhs=rwe_sb[:, k * 16:(k + 1) * 16],
                        start=(k == 0), stop=(k == KT - 1))
                nc.vector.tensor_tensor(out=gates_sb[:, ct:ct + 1],
                                        in0=psl[:, 0:1],
                                        in1=msk_sb[:, ct:ct + 1], op=MUL)

            # ---- w2 resident load (overlaps with B's compute) ----
            w2_sb = sres.tile([128, IT * H], dt_mm, tag="w2_sb", name="w2_sb")
            for i in range(IT):
                nc.sync.dma_start(w2_sb[:, i * H:(i + 1) * H],
                                  w2t[i * 128:(i + 1) * 128, :])

            # ---- Part B: expert ht[I, C] = silu(w1@x) * (w3@x) ----
            ht_sb = sres.tile([128, IT * C_], dt_mm, tag="ht_sb", name="ht_sb")
            for i in range(IT):
                w1b = wstr.tile([128, KT * 128], dt_mm, tag="w1b", name="w1b")
                nc.sync.dma_start(w1b[:],
                                  w1t[i].rearrange("(k p) m -> p k m", p=128))
                w3b = wstr.tile([128, KT * 128], dt_mm, tag="w3b", name="w3b")
                nc.sync.dma_start(w3b[:],
                                  w3t[i].rearrange("(k p) m -> p k m", p=128))
                for cc in range(n_ch):
                    psa = psA.tile([128, CH_], f32, tag="a", name="psa",
                                   space="PSUM")
                    psb = psB.tile([128, CH_], f32, tag="b", name="psb",
                                   space="PSUM")
                    for k in range(KT):
                        nc.tensor.matmul(
                            psa[:],
                            lhsT=w1b[:, k * 128:(k + 1) * 128],
                            rhs=xgb_sb[:, k * C_ + cc * CH_: k * C_ + (cc + 1) * CH_],
                            start=(k == 0), stop=(k == KT - 1))
                    for k in range(KT):
                        nc.tensor.matmul(
                            psb[:],
                            lhsT=w3b[:, k * 128:(k + 1) * 128],
                            rhs=xgb_sb[:, k * C_ + cc * CH_: k * C_ + (cc + 1) * CH_],
                            start=(k == 0), stop=(k == KT - 1))
                    sact = work.tile([128, CH_], f32, tag="sact", name="sact")
                    nc.scalar.activation(sact[:], psa[:], AF.Silu)
                    nc.vector.tensor_tensor(
                        out=ht_sb[:, i * C_ + cc * CH_: i * C_ + (cc + 1) * CH_],
                        in0=sact[:], in1=psb[:], op=MUL)

            # ---- Part C: expert y (gated) -> eacc, dense compact order ----
            for ct in range(CT_):
                ysb_c = work.tile([128, H], f32, tag="ysb_c", name="ysb_c",
                                  bufs=1 if lean else None)
                for hh in range(2):
                    psy = psY.tile([128, 512], f32, tag="y", name="psy",
                                   space="PSUM")
                    for i in range(IT):
                        nc.tensor.matmul(
                            psy[:],
                            lhsT=ht_sb[:, i * C_ + ct * 128: i * C_ + (ct + 1) * 128],
                            rhs=w2_sb[:, i * H + hh * 512: i * H + hh * 512 + 512],
                            start=(i == 0), stop=(i == IT - 1))
                    nc.scalar.activation(
                        ysb_c[:, hh * 512:(hh + 1) * 512],
                        psy[:], AF.Copy, scale=gates_sb[:, ct:ct + 1])
                nc.sync.dma_start(eacc[ct * 128:(ct + 1) * 128, :], ysb_c[:])

            # ---- D2: acc[t] = hts.T @ sw2_slice + eacc[inv_idx[t]] ----
            for trow in range(TT):
                geacc = work.tile([128, H], f32, tag="geacc",
                                  name="geacc", bufs=2 if lean else 3)
                nc.gpsimd.indirect_dma_start(
                    out=geacc[:], out_offset=None,
                    in_=eacc[:, :],
                    in_offset=bass.IndirectOffsetOnAxis(
                        ap=invi_sb[:, trow:trow + 1], axis=0))
                ysb = work.tile([128, H], f32, tag="ysb", name="ysb",
                                bufs=2 if lean else 3)
                for hh in range(2):
                    psy = psY.tile([128, 512], f32, tag="y", name="psy_s",
                                   space="PSUM")
                    nc.tensor.matmul(
                        psy[:],
                        lhsT=hts[:, trow * 128:(trow + 1) * 128],
                        rhs=s2_sb[:, hh * 512:(hh + 1) * 512],
                        start=True, stop=False)
                    nc.tensor.matmul(
                        psy[:],
                        lhsT=hts[:SIP[1], T + trow * 128: T + (trow + 1) * 128],
                        rhs=s2_sb[:SIP[1], H + hh * 512: H + (hh + 1) * 512],
                        start=False, stop=True)
                    nc.vector.tensor_add(ysb[:, hh * 512:(hh + 1) * 512],
                                         psy[:],
                                         geacc[:, hh * 512:(hh + 1) * 512])
                nc.sync.dma_start(acc[trow * 128:(trow + 1) * 128, :],
                                  ysb[:])

            # ---- Part E: cross-core combine + output ----
            if use_cc:
                nc.gpsimd.collective_compute(
                    "ReduceScatter",
                    mybir.AluOpType.add,
                    replica_groups=[list(range(NCORES))],
                    ins=[acc[:, :]],
                    outs=[rs_out[:, :]],
                )
                src_t = rs_out
            else:
                src_t = acc
            nc.sync.dma_start(out[:, :], src_t[0:TSL, :])

    nc.finalize()
    return nc


def _get_nc(reps=1):
    key = (reps, DTYPE, C)
    if key not in _BUILD_CACHE:
        _BUILD_CACHE[key] = _build(reps)
    return _BUILD_CACHE[key]


def _count_max(x2, router_w):
    logits = x2 @ router_w.T
    order = np.argsort(-logits, axis=1, kind="stable")[:, :TOPK]
    return max(int((order == e).any(axis=1).sum()) for e in range(E))


def _dispatch(x2, router_w, cap=None):
    """Host-side sharding decision: per-expert compact token lists."""
    cap = cap or C
    logits = x2 @ router_w.T                      # [T, E] fp32, dispatch only
    order = np.argsort(-logits, axis=1, kind="stable")[:, :TOPK]
    per_core = []
    all_rows = np.arange(T)
    for e in range(E):
        rows = all_rows[(order == e).any(axis=1)]
        ce = len(rows)
        assert ce <= cap, f"expert {e} overflow: {ce} > {cap}"
        unused = np.setdiff1d(all_rows, rows, assume_unique=True)
        pad = unused[:cap - ce]
        if len(pad) < cap - ce:   # cap > T - ce: reuse unused rows cyclically
            extra = np.resize(unused, cap - ce)
            pad = extra
        idx_full = np.concatenate([rows, pad]).astype(np.int32)
        mask = (np.arange(cap) < ce).astype(np.float32)
        inv = np.full(T, cap, dtype=np.int32)     # sentinel -> zero row
        inv[rows] = np.arange(ce, dtype=np.int32)
        per_core.append((idx_full, mask, inv))
    return per_core


def _make_in_maps(x2, router_w, w1, w2, w3, sw1, sw2, sw3, cap=None):
    if DTYPE == "bf16":
        import ml_dtypes
        np_mm = ml_dtypes.bfloat16
    else:
        np_mm = np.float32
    cap = cap or C
    dispatch = _dispatch(x2, router_w, cap)
    xt_host = np.ascontiguousarray(x2.T.astype(np_mm))
    in_maps = []
    for e in range(E):
        idx_full, mask, inv = dispatch[e]
        in_maps.append({
            "xg": np.ascontiguousarray(x2[idx_full].T),
            "xt": xt_host,
            "w1t": np.ascontiguousarray(
                np.asarray(w1[e], dtype=np.float32).reshape(IT, 128, H)
                .transpose(0, 2, 1).astype(np_mm)),
            "w3t": np.ascontiguousarray(
                np.asarray(w3[e], dtype=np.float32).reshape(IT, 128, H)
                .transpose(0, 2, 1).astype(np_mm)),
            "w2t": np.ascontiguousarray(
                np.asarray(w2[e], np.float32).T.astype(np_mm)),
            "s1t": np.ascontiguousarray(
                np.asarray(sw1[e * ILOC:(e + 1) * ILOC, :], np.float32)
                .T.astype(np_mm)),
            "s3t": np.ascontiguousarray(
                np.asarray(sw3[e * ILOC:(e + 1) * ILOC, :], np.float32)
                .T.astype(np_mm)),
            "s2t": np.ascontiguousarray(
                np.asarray(sw2[:, e * ILOC:(e + 1) * ILOC], np.float32)
                .T.astype(np_mm)),
            "rwe": np.ascontiguousarray(
                np.repeat(np.asarray(router_w[e], np.float32).reshape(H, 1),
                          16, axis=1)),
            "invi": inv.reshape(T, 1),
            "msk": np.ascontiguousarray(mask.reshape(cap // 128, 128).T),
        })
    return in_maps


def kernel(x, router_w, w1, w2, w3, sw1, sw2, sw3):
    from concourse.bass_utils import run_bass_kernel_spmd

    in_dtype = x.dtype
    x2 = np.ascontiguousarray(x.reshape(T, H), dtype=np.float32)
    router_w = np.asarray(router_w, dtype=np.float32)
    cap = C
    cmax = _count_max(x2, router_w)
    if cmax > C:   # unlikely re-routed inputs: rebuild with a larger capacity
        step = 256 if cmax <= 1024 else 512
        cap = -((-cmax) // step) * step
    key = (1, DTYPE, cap)
    if key not in _BUILD_CACHE:
        _BUILD_CACHE[key] = _build(1, cap=cap)
    nc = _BUILD_CACHE[key]

    in_maps = _make_in_maps(x2, router_w, w1, w2, w3, sw1, sw2, sw3, cap)
    res = run_bass_kernel_spmd(nc, in_maps, list(range(NCORES)))
    out = np.concatenate([res.results[i]["out"] for i in range(NCORES)],
                         axis=0)
    return out.reshape(x.shape).astype(in_dtype)


# revision 20
# speedup vs baseline: 571.6296x; 571.6296x over previous
"""MoE (8 experts, top-2, shared expert) Trainium2 kernel.

Expert-parallel over 8 NeuronCores. The host performs only the dispatch
decision (top-2 expert ids -> compact per-expert token lists) and data
layout (transposes/gathers); all floating-point model math — router
logits, gates, expert SwiGLU, shared expert, and the cross-core combine
(ReduceScatter) — runs on device in fp32r matmuls with fp32 accumulation.

Device program per core (SPMD, identical program, per-core data):
  D1: hts[176, T] = silu(sw1_slice @ x) * (sw3_slice @ x)  (all tokens)
  A:  router logits for compact tokens (matmul) * validity mask -> gates
  B:  ht[I, C] = silu(w1 @ xg) * (w3 @ xg)   (compact tokens)
  C:  y[ct] = gate * (ht.T @ w2t) -> eacc[C, H] (dense write, compact order)
  D2: acc[t] = hts.T @ sw2_slice + eacc[inv_idx[t]]   (indirect GATHER with
      zero-row sentinel for tokens not routed to this core)
  E:  ReduceScatter(add) over acc -> this core's 256-token output slice
"""

import numpy as np

H = 1024          # hidden
I = 1408          # moe intermediate
E = 8             # experts == cores
T = 2048          # tokens (2*1024)
TOPK = 2
C = 640           # compact per-expert token capacity (max observed 540)
CH = 320          # ht token chunk (2 chunks; >=256 keeps fp32r at full rate)
ILOC = I // E     # 176: shared-expert intermediate slice per core
TSL = T // E      # 256: output token slice per core
KT = H // 128     # 8 contraction tiles over H
IT = I // 128     # 11 tiles over I
CT = C // 128     # 5 compact token tiles
TT = T // 128     # 16 token tiles
SIP = (128, ILOC - 128)   # shared I-slice partition tiles: 128 + 48
NCORES = 8
DTYPE = "f32r"     # "f32r" (full precision-ish) or "bf16" (faster DMA)

_BUILD_CACHE = {}


def _build(reps=1, use_cc=True, dtype=None, cap=None):
    lean = cap is not None and cap > C
    import concourse.bacc as bacc
    import concourse.bass as bass
    import concourse.mybir as mybir
    from concourse import tile
    from contextlib import ExitStack

    f32 = mybir.dt.float32
    f32r = mybir.dt.float32r
    i32 = mybir.dt.int32
    dt_mm = mybir.dt.bfloat16 if (dtype or DTYPE) == "bf16" else f32r
    AF = mybir.ActivationFunctionType
    MUL = mybir.AluOpType.mult

    C_ = cap or C
    CT_ = C_ // 128
    n_ch = max(1, (C_ + 511) // 512)
    CH_ = C_ // n_ch
    assert CH_ * n_ch == C_ and CH_ % 64 == 0, (C_, CH_)

    nc = bacc.Bacc("TRN2", target_bir_lowering=False, debug=False,
                   num_devices=NCORES)

    xg = nc.declare_dram_parameter("xg", [H, C_], f32r, isOutput=False)
    xt = nc.declare_dram_parameter("xt", [H, T], dt_mm, isOutput=False)
    w1t = nc.declare_dram_parameter("w1t", [IT, H, 128], dt_mm, isOutput=False)
    w3t = nc.declare_dram_parameter("w3t", [IT, H, 128], dt_mm, isOutput=False)
    w2t = nc.declare_dram_parameter("w2t", [I, H], dt_mm, isOutput=False)
    s1t = nc.declare_dram_parameter("s1t", [H, ILOC], dt_mm, isOutput=False)
    s3t = nc.declare_dram_parameter("s3t", [H, ILOC], dt_mm, isOutput=False)
    s2t = nc.declare_dram_parameter("s2t", [ILOC, H], dt_mm, isOutput=False)
    rwe = nc.declare_dram_parameter("rwe", [H, 16], f32r, isOutput=False)
    invi = nc.declare_dram_parameter("invi", [T, 1], i32, isOutput=False)
    msk = nc.declare_dram_parameter("msk", [128, CT_], f32, isOutput=False)
    out = nc.declare_dram_parameter("out", [TSL, H], f32, isOutput=True)

    acc = nc.dram_tensor("acc", [T, H], f32)
    eacc = nc.dram_tensor("eacc", [C_ + 128, H], f32)
    rs_out = nc.dram_tensor("rs_out", [TSL, H], f32)

    with tile.TileContext(nc) as tc, ExitStack() as ctx:
        sres = ctx.enter_context(tc.tile_pool(name="sres", bufs=1))
        wstr = ctx.enter_context(tc.tile_pool(name="wstr",
                                              bufs=1 if lean else 2))
        xstr = ctx.enter_context(tc.tile_pool(name="xstr",
                                              bufs=1 if lean else 2))
        work = ctx.enter_context(tc.tile_pool(name="work", bufs=2))
        psA = ctx.enter_context(tc.tile_pool(name="psA", bufs=2, space="PSUM"))
        psB = ctx.enter_context(tc.tile_pool(name="psB", bufs=2, space="PSUM"))
        psY = ctx.enter_context(tc.tile_pool(name="psY", bufs=3, space="PSUM"))

        TCH = 256
        for _rep in range(reps):
            # ---- resident loads ----
            s13_sb = sres.tile([128, 2 * KT * ILOC], dt_mm, tag="s13_sb",
                               name="s13_sb")
            for k in range(KT):
                nc.sync.dma_start(s13_sb[:, k * ILOC:(k + 1) * ILOC],
                                  s1t[k * 128:(k + 1) * 128, :])
                nc.sync.dma_start(
                    s13_sb[:, (KT + k) * ILOC:(KT + k + 1) * ILOC],
                    s3t[k * 128:(k + 1) * 128, :])
            xg_sb = sres.tile([128, KT * C_], f32r, tag="xg_sb", name="xg_sb")
            for k in range(KT):
                nc.sync.dma_start(xg_sb[:, k * C_:(k + 1) * C_],
                                  xg[k * 128:(k + 1) * 128, :])
            rwe_sb = sres.tile([128, KT * 16], f32r, tag="rwe_sb",
                               name="rwe_sb")
            if dt_mm is f32r:
                xgb_sb = xg_sb
            else:
                xgb_sb = sres.tile([128, KT * C_], dt_mm, tag="xgb_sb",
                                   name="xgb_sb")
                for k in range(KT):
                    nc.vector.tensor_copy(xgb_sb[:, k * C_:(k + 1) * C_],
                                          xg_sb[:, k * C_:(k + 1) * C_])
            nc.sync.dma_start(rwe_sb[:],
                              rwe.rearrange("(k p) o -> p k o", p=128))
            invi_sb = sres.tile([128, TT], i32, tag="invi_sb", name="invi_sb")
            nc.sync.dma_start(invi_sb[:],
                              invi.rearrange("(c p) o -> p c o", p=128))
            msk_sb = sres.tile([128, CT_], f32, tag="msk_sb", name="msk_sb")
            nc.sync.dma_start(msk_sb[:], msk[:, :])
            s2_sb = sres.tile([128, 2 * H], dt_mm, tag="s2_sb", name="s2_sb")
            nc.sync.dma_start(s2_sb[:, 0:H], s2t[0:128, :])
            nc.sync.dma_start(s2_sb[:SIP[1], H:2 * H], s2t[128:ILOC, :])
            # zero sentinel row block for the combine gather
            ztile = work.tile([128, H], f32, tag="ztile", name="ztile",
                              bufs=1)
            nc.gpsimd.memset(ztile[:], 0.0)
            nc.sync.dma_start(eacc[C_:C_ + 128, :], ztile[:])

            # ---- D1: shared-expert hts[176, T] over all tokens ----
            hts = sres.tile([128, 2 * T], dt_mm, tag="hts", name="hts")
            for tt in range(T // TCH):
                xc = xstr.tile([128, KT * TCH], dt_mm, tag="xc", name="xc")
                for k in range(KT):
                    nc.sync.dma_start(
                        xc[:, k * TCH:(k + 1) * TCH],
                        xt[k * 128:(k + 1) * 128, tt * TCH:(tt + 1) * TCH])
                for si in range(2):
                    sip = SIP[si]
                    psa = psA.tile([128, TCH], f32, tag="a", name="psa_s",
                                   space="PSUM")
                    psb = psB.tile([128, TCH], f32, tag="b", name="psb_s",
                                   space="PSUM")
                    for k in range(KT):
                        nc.tensor.matmul(
                            psa[:sip, :],
                            lhsT=s13_sb[:, k * ILOC + si * 128:
                                        k * ILOC + si * 128 + sip],
                            rhs=xc[:, k * TCH:(k + 1) * TCH],
                            start=(k == 0), stop=(k == KT - 1))
                    for k in range(KT):
                        nc.tensor.matmul(
                            psb[:sip, :],
                            lhsT=s13_sb[:, (KT + k) * ILOC + si * 128:
                                        (KT + k) * ILOC + si * 128 + sip],
                            rhs=xc[:, k * TCH:(k + 1) * TCH],
                            start=(k == 0), stop=(k == KT - 1))
                    sact = work.tile([128, TCH], f32, tag="sact_s",
                                     name="sact_s",
                                     bufs=1 if lean else None)
                    nc.scalar.activation(sact[:sip, :], psa[:sip, :], AF.Silu)
                    nc.vector.tensor_tensor(
                        out=hts[:sip, si * T + tt * TCH:
                                si * T + (tt + 1) * TCH],
                        in0=sact[:sip, :], in1=psb[:sip, :], op=MUL)

            # ---- Part A: logits for compact tokens -> gates ----
            gates_sb = sres.tile([128, CT_], f32, tag="gates_sb",
                                 name="gates_sb")
            for ct in range(CT_):
                psl = psY.tile([128, 512], f32, tag="y", name="psl",
                               space="PSUM")
                for k in range(KT):
                    nc.tensor.matmul(
                        psl[:, 0:16],
                        lhsT=xg_sb[:, k * C_ + ct * 128: k * C_ + (ct + 1) * 128],
                        rhs=rwe_sb[:, k * 16:(k + 1) * 16],
                        start=(k == 0), stop=(k == KT - 1))
                nc.vector.tensor_tensor(out=gates_sb[:, ct:ct + 1],
                                        in0=psl[:, 0:1],
                                        in1=msk_sb[:, ct:ct + 1], op=MUL)

            # ---- w2 resident load (overlaps with B's compute) ----
            w2_sb = sres.tile([128, IT * H], dt_mm, tag="w2_sb", name="w2_sb")
            for i in range(IT):
                nc.sync.dma_start(w2_sb[:, i * H:(i + 1) * H],
                                  w2t[i * 128:(i + 1) * 128, :])

            # ---- Part B: expert ht[I, C] = silu(w1@x) * (w3@x) ----
            ht_sb = sres.tile([128, IT * C_], dt_mm, tag="ht_sb", name="ht_sb")
            for i in range(IT):
                w1b = wstr.tile([128, KT * 128], dt_mm, tag="w1b", name="w1b")
                nc.sync.dma_start(w1b[:],
                                  w1t[i].rearrange("(k p) m -> p k m", p=128))
                w3b = wstr.tile([128, KT * 128], dt_mm, tag="w3b", name="w3b")
                nc.sync.dma_start(w3b[:],
                                  w3t[i].rearrange("(k p) m -> p k m", p=128))
                for cc in range(n_ch):
                    psa = psA.tile([128, CH_], f32, tag="a", name="psa",
                                   space="PSUM")
                    psb = psB.tile([128, CH_], f32, tag="b", name="psb",
                                   space="PSUM")
                    for k in range(KT):
                        nc.tensor.matmul(
                            psa[:],
                            lhsT=w1b[:, k * 128:(k + 1) * 128],
                            rhs=xgb_sb[:, k * C_ + cc * CH_: k * C_ + (cc + 1) * CH_],
                            start=(k == 0), stop=(k == KT - 1))
                    for k in range(KT):
                        nc.tensor.matmul(
                            psb[:],
                            lhsT=w3b[:, k * 128:(k + 1) * 128],
                            rhs=xgb_sb[:, k * C_ + cc * CH_: k * C_ + (cc + 1) * CH_],
                            start=(k == 0), stop=(k == KT - 1))
                    sact = work.tile([128, CH_], f32, tag="sact", name="sact")
                    nc.scalar.activation(sact[:], psa[:], AF.Silu)
                    nc.vector.tensor_tensor(
                        out=ht_sb[:, i * C_ + cc * CH_: i * C_ + (cc + 1) * CH_],
                        in0=sact[:], in1=psb[:], op=MUL)

            # ---- Part C: expert y (gated) -> eacc, dense compact order ----
            for ct in range(CT_):
                ysb_c = work.tile([128, H], f32, tag="ysb_c", name="ysb_c",
                                  bufs=1 if lean else None)
                for hh in range(2):
                    psy = psY.tile([128, 512], f32, tag="y", name="psy",
                                   space="PSUM")
                    for i in range(IT):
                        nc.tensor.matmul(
                            psy[:],
                            lhsT=ht_sb[:, i * C_ + ct * 128: i * C_ + (ct + 1) * 128],
                            rhs=w2_sb[:, i * H + hh * 512: i * H + hh * 512 + 512],
                            start=(i == 0), stop=(i == IT - 1))
                    nc.scalar.activation(
                        ysb_c[:, hh * 512:(hh + 1) * 512],
                        psy[:], AF.Copy, scale=gates_sb[:, ct:ct + 1])
                nc.sync.dma_start(eacc[ct * 128:(ct + 1) * 128, :], ysb_c[:])

            # ---- D2: acc[t] = hts.T @ sw2_slice + eacc[inv_idx[t]] ----
            for trow in range(TT):
                geacc = work.tile([128, H], f32, tag="geacc",
                                  name="geacc", bufs=2 if lean else 3)
                nc.gpsimd.indirect_dma_start(
                    out=geacc[:], out_offset=None,
                    in_=eacc[:, :],
                    in_offset=bass.IndirectOffsetOnAxis(
                        ap=invi_sb[:, trow:trow + 1], axis=0))
                ysb = work.tile([128, H], f32, tag="ysb", name="ysb",
                                bufs=2 if lean else 3)
                for hh in range(2):
                    psy = psY.tile([128, 512], f32, tag="y", name="psy_s",
                                   space="PSUM")
                    nc.tensor.matmul(
                        psy[:],
                        lhsT=hts[:, trow * 128:(trow + 1) * 128],
                        rhs=s2_sb[:, hh * 512:(hh + 1) * 512],
                        start=True, stop=False)
                    nc.tensor.matmul(
                        psy[:],
                        lhsT=hts[:SIP[1], T + trow * 128: T + (trow + 1) * 128],
                        rhs=s2_sb[:SIP[1], H + hh * 512: H + (hh + 1) * 512],
                        start=False, stop=True)
                    nc.vector.tensor_add(ysb[:, hh * 512:(hh + 1) * 512],
                                         psy[:],
                                         geacc[:, hh * 512:(hh + 1) * 512])
                nc.sync.dma_start(acc[trow * 128:(trow + 1) * 128, :],
                                  ysb[:])

            # ---- Part E: cross-core combine + output ----
            # (A 2-way split RS overlapped with D2's tail models WORSE:
            # 266.8us vs 263.2us — D2's tail is too short to hide a
            # collective and the extra launch overhead nets a loss.)
            if use_cc:
                nc.gpsimd.collective_compute(
                    "ReduceScatter",
                    mybir.AluOpType.add,
                    replica_groups=[list(range(NCORES))],
                    ins=[acc[:, :]],
                    outs=[rs_out[:, :]],
                )
                src_t = rs_out
            else:
                src_t = acc
            nc.sync.dma_start(out[:, :], src_t[0:TSL, :])

    nc.finalize()
    return nc


def _get_nc(reps=1):
    key = (reps, DTYPE, C)
    if key not in _BUILD_CACHE:
        _BUILD_CACHE[key] = _build(reps)
    return _BUILD_CACHE[key]


def _count_max(x2, router_w):
    logits = x2 @ router_w.T
    order = np.argsort(-logits, axis=1, kind="stable")[:, :TOPK]
    return max(int((order == e).any(axis=1).sum()) for e in range(E))


def _dispatch(x2, router_w, cap=None):
    """Host-side sharding decision: per-expert compact token lists."""
    cap = cap or C
    logits = x2 @ router_w.T                      # [T, E] fp32, dispatch only
    order = np.argsort(-logits, axis=1, kind="stable")[:, :TOPK]
    per_core = []
    all_rows = np.arange(T)
    for e in range(E):
        rows = all_rows[(order == e).any(axis=1)]
        ce = len(rows)
        assert ce <= cap, f"expert {e} overflow: {ce} > {cap}"
        unused = np.setdiff1d(all_rows, rows, assume_unique=True)
        pad = unused[:cap - ce]
        if len(pad) < cap - ce:   # cap > T - ce: reuse unused rows cyclically
            extra = np.resize(unused, cap - ce)
            pad = extra
        idx_full = np.concatenate([rows, pad]).astype(np.int32)
        mask = (np.arange(cap) < ce).astype(np.float32)
        inv = np.full(T, cap, dtype=np.int32)     # sentinel -> zero row
        inv[rows] = np.arange(ce, dtype=np.int32)
        per_core.append((idx_full, mask, inv))
    return per_core


def _make_in_maps(x2, router_w, w1, w2, w3, sw1, sw2, sw3, cap=None):
    if DTYPE == "bf16":
        import ml_dtypes
        np_mm = ml_dtypes.bfloat16
    else:
        np_mm = np.float32
    cap = cap or C
    dispatch = _dispatch(x2, router_w, cap)
    xt_host = np.ascontiguousarray(x2.T.astype(np_mm))
    in_maps = []
    for e in range(E):
        idx_full, mask, inv = dispatch[e]
        in_maps.append({
            "xg": np.ascontiguousarray(x2[idx_full].T),
            "xt": xt_host,
            "w1t": np.ascontiguousarray(
                np.asarray(w1[e], dtype=np.float32).reshape(IT, 128, H)
                .transpose(0, 2, 1).astype(np_mm)),
            "w3t": np.ascontiguousarray(
                np.asarray(w3[e], dtype=np.float32).reshape(IT, 128, H)
                .transpose(0, 2, 1).astype(np_mm)),
            "w2t": np.ascontiguousarray(
                np.asarray(w2[e], np.float32).T.astype(np_mm)),
            "s1t": np.ascontiguousarray(
                np.asarray(sw1[e * ILOC:(e + 1) * ILOC, :], np.float32)
                .T.astype(np_mm)),
            "s3t": np.ascontiguousarray(
                np.asarray(sw3[e * ILOC:(e + 1) * ILOC, :], np.float32)
                .T.astype(np_mm)),
            "s2t": np.ascontiguousarray(
                np.asarray(sw2[:, e * ILOC:(e + 1) * ILOC], np.float32)
                .T.astype(np_mm)),
            "rwe": np.ascontiguousarray(
                np.repeat(np.asarray(router_w[e], np.float32).reshape(H, 1),
                          16, axis=1)),
            "invi": inv.reshape(T, 1),
            "msk": np.ascontiguousarray(mask.reshape(cap // 128, 128).T),
        })
    return in_maps


def kernel(x, router_w, w1, w2, w3, sw1, sw2, sw3):
    from concourse.bass_utils import run_bass_kernel_spmd

    in_dtype = x.dtype
    x2 = np.ascontiguousarray(x.reshape(T, H), dtype=np.float32)
    router_w = np.asarray(router_w, dtype=np.float32)
    cap = C
    cmax = _count_max(x2, router_w)
    if cmax > C:   # unlikely re-routed inputs: rebuild with a larger capacity
        step = 256 if cmax <= 1024 else 512
        cap = -((-cmax) // step) * step
    key = (1, DTYPE, cap)
    if key not in _BUILD_CACHE:
        _BUILD_CACHE[key] = _build(1, cap=cap)
    nc = _BUILD_CACHE[key]

    in_maps = _make_in_maps(x2, router_w, w1, w2, w3, sw1, sw2, sw3, cap)
    res = run_bass_kernel_spmd(nc, in_maps, list(range(NCORES)))
    out = np.concatenate([res.results[i]["out"] for i in range(NCORES)],
                         axis=0)
    return out.reshape(x.shape).astype(in_dtype)
